# revision 4
# baseline (speedup 1.0000x reference)
"""Trainium2 Bass kernel for BitLinear: y[b,s,o] = sum_d x[b,s,d] * w[o,d].

x: [4, 2048, 4096] f32, weight: [4096, 4096] int32 (values 0..255), y f32.

Strategy:
- Data-parallel over tokens: 8192 tokens -> 8 cores x 1024 tokens.
- Precision: single bf16 pass. Weight values 0..255 are EXACT in bf16
  (8-bit mantissa), so the only error is x's bf16 quantization (~2^-9
  rel) -> max rel err ~2e-3, well under the 2e-2 gate, at half the
  matmul work of a hi/lo split.
- W-stationary formulation: out yt[n, m] = W^T[k, n]^T @ X^T[k, m].
  Each 128-column stationary weight load feeds the two 512-token moving
  chunks -> 2 consecutive matmuls per load.
- X^T shard (8 MB bf16) is streamed into SBUF during the first output
  group and stays resident; W^T (32 MB, single copy) streams per k-tile.
- Host gathers per-core yt [4096, 1024] f32, transposes, concatenates.
"""

import sys

for _p in ("/opt/trn_rl_repo", "/root/.axon_site/_ro/trn_rl_repo"):
    if _p not in sys.path:
        sys.path.append(_p)

import numpy as np
import ml_dtypes

N_CORES = 8
TOKENS = 8192  # 4 * 2048
D_IN = 4096
D_OUT = 4096
T_SHARD = TOKENS // N_CORES  # 1024

_NC_CACHE = {}


def build_nc(repeats: int = 1):
    """Build (and cache) the Bass program.

    repeats > 1 re-emits the compute body (used only for slope-based HW
    timing; identical output)."""
    if repeats in _NC_CACHE:
        return _NC_CACHE[repeats]

    import concourse.mybir as mybir
    import concourse.tile as tile
    from concourse import bacc

    P = 128
    nc = bacc.Bacc(None, target_bir_lowering=False)
    with tile.TileContext(nc) as tc:
        with tc.tile_pool(name="dram", bufs=1, space="DRAM") as dram:
            kxm = dram.tile([D_IN, T_SHARD], mybir.dt.bfloat16,
                            kind="ExternalInput", name="kxm", uniquify=False)
            kxns = dram.tile([D_IN, D_OUT], mybir.dt.bfloat16,
                             kind="ExternalInput", name="kxns", uniquify=False)
            yt = dram.tile([D_OUT, T_SHARD], mybir.dt.float32,
                           kind="ExternalOutput", name="yt", uniquify=False)
            kxm3 = kxm[:].rearrange("(ko p) m -> p ko m", p=P)  # [128, 32, 1024]
            with tc.tile_pool(name="xpool", bufs=32) as xpool, \
                 tc.tile_pool(name="wpool", bufs=4) as wpool, \
                 tc.tile_pool(name="pspool", bufs=2, space="PSUM") as pspool, \
                 tc.tile_pool(name="evpool", bufs=4) as evpool:
                xtiles = [None] * 32
                NG = D_OUT // 256   # 16 groups of 256 output features
                KT = D_IN // P      # 32 k-tiles
                MC = T_SHARD // 512  # 2 moving chunks of 512 tokens
                first = True
                for _ in range(repeats):
                    for ng in range(NG):
                        banks = {}
                        for nsl in range(2):
                            for mc in range(MC):
                                banks[(nsl, mc)] = pspool.tile(
                                    [P, 512], mybir.dt.float32,
                                    name=f"bank_{nsl}_{mc}",
                                    tag=f"bank_{nsl}_{mc}")
                        for k in range(KT):
                            wt = wpool.tile([P, 256], mybir.dt.bfloat16,
                                            name="wt", tag="wt")
                            nc.sync.dma_start(
                                wt[:], kxns[k * P:(k + 1) * P,
                                            ng * 256:(ng + 1) * 256])
                            if first:  # JIT-load the xtile so X streaming
                                # hides under ng=0 compute
                                xt = xpool.tile([P, T_SHARD],
                                                mybir.dt.bfloat16,
                                                name="xt", tag="xt")
                                nc.sync.dma_start(xt[:], kxm3[:, k])
                                xtiles[k] = xt
                            for nsl in range(2):
                                lhsT = wt[:, nsl * P:(nsl + 1) * P]
                                # 4 moving chunks of 256 per stationary load,
                                # consecutive pairs on the same PSUM bank
                                # (mirrors the load-hiding pattern measured
                                # fully hidden in the hi/lo baseline)
                                for mc in range(MC):
                                    for h in range(2):
                                        nc.tensor.matmul(
                                            banks[(nsl, mc)][
                                                :, h * 256:(h + 1) * 256],
                                            lhsT,
                                            xtiles[k][
                                                :, mc * 512 + h * 256:
                                                mc * 512 + (h + 1) * 256],
                                            # start zeroes the WHOLE 2KB
                                            # bank row, so only h==0 may
                                            # start; h==1 accumulates onto
                                            # the zeroed region
                                            start=(k == 0 and h == 0),
                                            stop=(k == KT - 1 and h == 1),
                                        )
                        first = False
                        for nsl in range(2):
                            for mc in range(MC):
                                ev = evpool.tile([P, 512], mybir.dt.float32,
                                                 name="ev", tag="ev")
                                nc.vector.tensor_copy(
                                    out=ev[:], in_=banks[(nsl, mc)][:])
                                nc.sync.dma_start(
                                    yt[ng * 256 + nsl * P:
                                       ng * 256 + (nsl + 1) * P,
                                       mc * 512:(mc + 1) * 512],
                                    ev[:])
    nc.compile()
    _NC_CACHE[repeats] = nc
    return nc


def prepare_in_maps(x: np.ndarray, weight: np.ndarray):
    """Host-side shard prep: bf16 x^T shards; W^T as a single bf16 copy
    (values 0..255 are exact in bf16)."""
    bf16 = ml_dtypes.bfloat16
    x2 = np.ascontiguousarray(np.asarray(x).reshape(TOKENS, D_IN))
    kxm_full = np.ascontiguousarray(x2.astype(bf16).T)  # [D_IN, TOKENS]

    wt = np.ascontiguousarray(
        np.asarray(weight).astype(np.float32).astype(bf16).T)  # [D_IN, D_OUT]

    in_maps = []
    for c in range(N_CORES):
        kxm_c = np.ascontiguousarray(
            kxm_full[:, c * T_SHARD:(c + 1) * T_SHARD])
        in_maps.append({"kxm": kxm_c, "kxns": wt})
    return in_maps


def gather_output(results):
    y = np.concatenate(
        [np.ascontiguousarray(results[c]["yt"].T) for c in range(N_CORES)],
        axis=0)
    return y.reshape(4, 2048, D_OUT).astype(np.float32, copy=False)


def kernel(x: np.ndarray, weight: np.ndarray) -> np.ndarray:
    from concourse.bass_utils import run_bass_kernel_spmd

    nc = build_nc()
    in_maps = prepare_in_maps(x, weight)
    res = run_bass_kernel_spmd(nc, in_maps, core_ids=list(range(N_CORES)))
    return gather_output(res.results)


# revision 7
# speedup vs baseline: 1.2610x; 1.2610x over previous
"""Trainium2 Bass kernel for BitLinear: y[b,s,o] = sum_d x[b,s,d] * w[o,d].

x: [4, 2048, 4096] f32, weight: [4096, 4096] int32 (values 0..255), y f32.

Strategy:
- Data-parallel over tokens: 8192 tokens -> 8 cores x 1024 tokens.
- Precision: SINGLE bf16 pass. Weight values 0..255 are exact in bf16
  (8-bit mantissa), so the only error is x's bf16 quantization (~2^-9
  rel): max rel err ~1.9e-3 against the 2e-2 gate, at half the matmul
  work of a hi/lo split. (fp8 DoubleRow was measured at 2x FLOP rate =
  157 TF/s, but the accuracy-required 3-term scheme needs 1.5x the
  bf16 single-pass time -> strictly worse.)
- W-stationary: yt[n, m] = W^T[k, n]^T @ X^T[k, m]; each 128-col
  stationary load feeds 2 moving M=512 matmuls (the two 512-token PSUM
  banks). 2048 MMs/core -> ~437 us back-to-back at 2.4 GHz; measured
  PE occupancy ~96% in TimelineSim.
- W is host-pre-tiled to [16, 128, 8192] so each output group's
  weights arrive in ONE contiguous 2MB DMA (16 weight DMAs per pass,
  not 512 -- per-DMA dispatch overhead dominated the un-batched
  version), double-buffered under the previous group's compute. The
  FIRST group of the first pass instead loads per-k 64KB slices so the
  PE starts ~2 us in rather than waiting for the full slab.
- X^T shard (8 MB bf16) streams in during group 0 and stays resident.
- Evictions: per 128-row slice, 2 PSUM banks -> one [128,1024] SBUF
  tile -> one DMA (64 vector copies + 32 output DMAs per pass).
- Host gathers per-core yt [4096, 1024] f32, transposes, concatenates.
"""

import sys

for _p in ("/opt/trn_rl_repo", "/root/.axon_site/_ro/trn_rl_repo"):
    if _p not in sys.path:
        sys.path.append(_p)

import numpy as np
import ml_dtypes

N_CORES = 8
TOKENS = 8192
D_IN = 4096
D_OUT = 4096
T_SHARD = TOKENS // N_CORES  # 1024
NG = D_OUT // 256  # 16 output groups

_NC_CACHE = {}


def build_nc(repeats: int = 1):
    """Build (and cache) the Bass program.

    repeats > 1 re-emits the compute body (used only for slope-based HW
    timing; identical output)."""
    if repeats in _NC_CACHE:
        return _NC_CACHE[repeats]

    import concourse.mybir as mybir
    import concourse.tile as tile
    from concourse import bacc

    P = 128
    KT = D_IN // P  # 32
    nc = bacc.Bacc(None, target_bir_lowering=False)
    with tile.TileContext(nc) as tc:
        with tc.tile_pool(name="dram", bufs=1, space="DRAM") as dram:
            kxm = dram.tile([D_IN, T_SHARD], mybir.dt.bfloat16,
                            kind="ExternalInput", name="kxm", uniquify=False)
            kxns = dram.tile([NG, P, KT * 256], mybir.dt.bfloat16,
                             kind="ExternalInput", name="kxns", uniquify=False)
            yt = dram.tile([D_OUT, T_SHARD], mybir.dt.float32,
                           kind="ExternalOutput", name="yt", uniquify=False)
            kxm3 = kxm[:].rearrange("(ko p) m -> p ko m", p=P)  # [128,32,1024]
            with tc.tile_pool(name="xpool", bufs=32) as xpool, \
                 tc.tile_pool(name="wpool", bufs=2) as wpool, \
                 tc.tile_pool(name="pspool", bufs=2, space="PSUM") as pspool, \
                 tc.tile_pool(name="evpool", bufs=4) as evpool:
                xtiles = [None] * KT
                first = True
                for _ in range(repeats):
                    for ng in range(NG):
                        wt = wpool.tile([P, KT * 256], mybir.dt.bfloat16,
                                        name="wt", tag="wt")
                        if not first:
                            nc.sync.dma_start(wt[:], kxns[ng])
                        else:
                            # fast start: 4 W chunks so the PE can begin
                            # after ~1/4 of the slab instead of all of it
                            for q in range(4):
                                nc.sync.dma_start(
                                    wt[:, q * 8 * 256:(q + 1) * 8 * 256],
                                    kxns[ng][:, q * 8 * 256:(q + 1) * 8 * 256])
                        banks = {}
                        for nsl in range(2):
                            for mc in range(2):
                                banks[(nsl, mc)] = pspool.tile(
                                    [P, 512], mybir.dt.float32,
                                    name=f"bank_{nsl}_{mc}",
                                    tag=f"bank_{nsl}_{mc}")
                        for k in range(KT):
                            if first:
                                # JIT x tiles: X streaming hides under
                                # group-0 compute and stays resident
                                xt = xpool.tile([P, T_SHARD],
                                                mybir.dt.bfloat16,
                                                name="xt", tag="xt")
                                nc.sync.dma_start(xt[:], kxm3[:, k])
                                xtiles[k] = xt
                            for nsl in range(2):
                                lhsT = wt[:, k * 256 + nsl * P:
                                          k * 256 + (nsl + 1) * P]
                                for mc in range(2):
                                    nc.tensor.matmul(
                                        banks[(nsl, mc)][:],
                                        lhsT,
                                        xtiles[k][:, mc * 512:(mc + 1) * 512],
                                        start=(k == 0),
                                        stop=(k == KT - 1),
                                    )
                        first = False
                        for nsl in range(2):
                            ev = evpool.tile([P, T_SHARD], mybir.dt.float32,
                                             name="ev", tag="ev")
                            for mc in range(2):
                                nc.vector.tensor_copy(
                                    out=ev[:, mc * 512:(mc + 1) * 512],
                                    in_=banks[(nsl, mc)][:])
                            nc.sync.dma_start(
                                yt[ng * 256 + nsl * P:
                                   ng * 256 + (nsl + 1) * P, :],
                                ev[:])
    nc.compile()
    _NC_CACHE[repeats] = nc
    return nc


def prepare_in_maps(x: np.ndarray, weight: np.ndarray):
    """Host-side prep: bf16 x^T shards; W^T pre-tiled to [16, 128, 8192]
    (one contiguous slab per output group; values 0..255 exact in bf16)."""
    bf16 = ml_dtypes.bfloat16
    x2 = np.ascontiguousarray(np.asarray(x).reshape(TOKENS, D_IN))
    kxm_full = np.ascontiguousarray(x2.astype(bf16).T)  # [D_IN, TOKENS]

    wt = np.asarray(weight).astype(np.float32).astype(bf16).T  # [D_IN, D_OUT]
    wt = wt.reshape(32, 128, NG, 256).transpose(2, 1, 0, 3)
    kxns = np.ascontiguousarray(wt.reshape(NG, 128, 32 * 256))

    in_maps = []
    for c in range(N_CORES):
        kxm_c = np.ascontiguousarray(
            kxm_full[:, c * T_SHARD:(c + 1) * T_SHARD])
        in_maps.append({"kxm": kxm_c, "kxns": kxns})
    return in_maps


def gather_output(results):
    y = np.concatenate(
        [np.ascontiguousarray(results[c]["yt"].T) for c in range(N_CORES)],
        axis=0)
    return y.reshape(4, 2048, D_OUT).astype(np.float32, copy=False)


def kernel(x: np.ndarray, weight: np.ndarray) -> np.ndarray:
    from concourse.bass_utils import run_bass_kernel_spmd

    nc = build_nc()
    in_maps = prepare_in_maps(x, weight)
    res = run_bass_kernel_spmd(nc, in_maps, core_ids=list(range(N_CORES)))
    return gather_output(res.results)


# revision 10
# speedup vs baseline: 1.3289x; 1.0538x over previous
"""Trainium2 Bass kernel for BitLinear: y[b,s,o] = sum_d x[b,s,d] * w[o,d].

x: [4, 2048, 4096] f32, weight: [4096, 4096] int32 (values 0..255), y f32.

Strategy:
- Data-parallel over tokens: 8192 tokens -> 8 cores x 1024 tokens.
- Precision: SINGLE bf16 pass. Weight values 0..255 are exact in bf16
  (8-bit mantissa), so the only error is x's bf16 quantization (~2^-9
  rel): max rel err ~1.9e-3 against the 2e-2 gate, at half the matmul
  work of a hi/lo split. (fp8 DoubleRow was measured at 2x FLOP rate =
  157 TF/s, but the accuracy-required 3-term scheme needs 1.5x the
  bf16 single-pass time -> strictly worse.)
- W-stationary: yt[n, m] = W^T[k, n]^T @ X^T[k, m]; each 128-col
  stationary load feeds 2 moving M=512 matmuls (the two 512-token PSUM
  banks). 2048 MMs/core -> ~437 us back-to-back at 2.4 GHz; measured
  PE occupancy ~96% in TimelineSim.
- W is host-pre-tiled to [16, 128, 8192] so each output group's
  weights arrive in ONE contiguous 2MB DMA (16 weight DMAs per pass,
  not 512 -- per-DMA dispatch overhead dominated the un-batched
  version), double-buffered under the previous group's compute. The
  FIRST group of the first pass instead loads per-k 64KB slices so the
  PE starts ~2 us in rather than waiting for the full slab.
- X^T shard (8 MB bf16) streams in during group 0 and stays resident.
- Evictions: per 128-row slice, 2 PSUM banks -> one [128,1024] SBUF
  tile -> one DMA (64 vector copies + 32 output DMAs per pass).
- Host gathers per-core yt [4096, 1024] f32, transposes, concatenates.
"""

import sys

for _p in ("/opt/trn_rl_repo", "/root/.axon_site/_ro/trn_rl_repo"):
    if _p not in sys.path:
        sys.path.append(_p)

import numpy as np
import ml_dtypes

N_CORES = 8
TOKENS = 8192
D_IN = 4096
D_OUT = 4096
T_SHARD = TOKENS // N_CORES  # 1024
NG = D_OUT // 256  # 16 output groups

_NC_CACHE = {}


def build_nc(repeats: int = 1):
    """Build (and cache) the Bass program.

    repeats > 1 re-emits the compute body (used only for slope-based HW
    timing; identical output)."""
    if repeats in _NC_CACHE:
        return _NC_CACHE[repeats]

    import concourse.mybir as mybir
    import concourse.tile as tile
    from concourse import bacc

    P = 128
    KT = D_IN // P  # 32
    nc = bacc.Bacc(None, target_bir_lowering=False)
    with tile.TileContext(nc) as tc:
        with tc.tile_pool(name="dram", bufs=1, space="DRAM") as dram:
            kxm = dram.tile([D_IN, T_SHARD], mybir.dt.bfloat16,
                            kind="ExternalInput", name="kxm", uniquify=False)
            kxns = dram.tile([NG, P, KT * 256], mybir.dt.bfloat16,
                             kind="ExternalInput", name="kxns", uniquify=False)
            yt = dram.tile([D_OUT, T_SHARD], mybir.dt.float32,
                           kind="ExternalOutput", name="yt", uniquify=False)
            kxm3 = kxm[:].rearrange("(ko p) m -> p ko m", p=P)  # [128,32,1024]
            with tc.tile_pool(name="xpool", bufs=32) as xpool, \
                 tc.tile_pool(name="wpool", bufs=2) as wpool, \
                 tc.tile_pool(name="pspool", bufs=2, space="PSUM") as pspool, \
                 tc.tile_pool(name="evpool", bufs=4) as evpool:
                xtiles = [None] * KT
                first = True
                for _ in range(repeats):
                    for ng in range(NG):
                        wt = wpool.tile([P, KT * 256], mybir.dt.bfloat16,
                                        name="wt", tag="wt")
                        if not first:
                            nc.sync.dma_start(wt[:], kxns[ng])
                        else:
                            # fast start: 4 W chunks so the PE can begin
                            # after ~1/4 of the slab instead of all of it
                            for q in range(4):
                                nc.sync.dma_start(
                                    wt[:, q * 8 * 256:(q + 1) * 8 * 256],
                                    kxns[ng][:, q * 8 * 256:(q + 1) * 8 * 256])
                        banks = {}
                        for nsl in range(2):
                            for mc in range(2):
                                banks[(nsl, mc)] = pspool.tile(
                                    [P, 512], mybir.dt.float32,
                                    name=f"bank_{nsl}_{mc}",
                                    tag=f"bank_{nsl}_{mc}")
                        for k in range(KT):
                            if first:
                                # JIT x tiles: X streaming hides under
                                # group-0 compute and stays resident
                                xt = xpool.tile([P, T_SHARD],
                                                mybir.dt.bfloat16,
                                                name="xt", tag="xt")
                                # Activation-engine DGE queue: x streams
                                # concurrently with W on SP's queue
                                nc.scalar.dma_start(xt[:], kxm3[:, k])
                                xtiles[k] = xt
                            for nsl in range(2):
                                lhsT = wt[:, k * 256 + nsl * P:
                                          k * 256 + (nsl + 1) * P]
                                for mc in range(2):
                                    nc.tensor.matmul(
                                        banks[(nsl, mc)][:],
                                        lhsT,
                                        xtiles[k][:, mc * 512:(mc + 1) * 512],
                                        start=(k == 0),
                                        stop=(k == KT - 1),
                                    )
                        first = False
                        if ng == NG - 1:
                            # last group: per-bank eviction so each DMA
                            # starts right after its bank's stop-MM,
                            # shortening the end-of-kernel drain
                            for nsl in range(2):
                                for mc in range(2):
                                    ev = evpool.tile(
                                        [P, 512], mybir.dt.float32,
                                        name="evs", tag="evs")
                                    nc.vector.tensor_copy(
                                        out=ev[:], in_=banks[(nsl, mc)][:])
                                    nc.scalar.dma_start(
                                        yt[ng * 256 + nsl * P:
                                           ng * 256 + (nsl + 1) * P,
                                           mc * 512:(mc + 1) * 512],
                                        ev[:])
                        else:
                            for nsl in range(2):
                                ev = evpool.tile(
                                    [P, T_SHARD], mybir.dt.float32,
                                    name="ev", tag="ev")
                                for mc in range(2):
                                    nc.vector.tensor_copy(
                                        out=ev[:, mc * 512:(mc + 1) * 512],
                                        in_=banks[(nsl, mc)][:])
                                nc.scalar.dma_start(
                                    yt[ng * 256 + nsl * P:
                                       ng * 256 + (nsl + 1) * P, :],
                                    ev[:])
    nc.compile()
    _NC_CACHE[repeats] = nc
    return nc


def prepare_in_maps(x: np.ndarray, weight: np.ndarray):
    """Host-side prep: bf16 x^T shards; W^T pre-tiled to [16, 128, 8192]
    (one contiguous slab per output group; values 0..255 exact in bf16)."""
    bf16 = ml_dtypes.bfloat16
    x2 = np.ascontiguousarray(np.asarray(x).reshape(TOKENS, D_IN))
    kxm_full = np.ascontiguousarray(x2.astype(bf16).T)  # [D_IN, TOKENS]

    wt = np.asarray(weight).astype(np.float32).astype(bf16).T  # [D_IN, D_OUT]
    wt = wt.reshape(32, 128, NG, 256).transpose(2, 1, 0, 3)
    kxns = np.ascontiguousarray(wt.reshape(NG, 128, 32 * 256))

    in_maps = []
    for c in range(N_CORES):
        kxm_c = np.ascontiguousarray(
            kxm_full[:, c * T_SHARD:(c + 1) * T_SHARD])
        in_maps.append({"kxm": kxm_c, "kxns": kxns})
    return in_maps


def gather_output(results):
    y = np.concatenate(
        [np.ascontiguousarray(results[c]["yt"].T) for c in range(N_CORES)],
        axis=0)
    return y.reshape(4, 2048, D_OUT).astype(np.float32, copy=False)


def kernel(x: np.ndarray, weight: np.ndarray) -> np.ndarray:
    from concourse.bass_utils import run_bass_kernel_spmd

    nc = build_nc()
    in_maps = prepare_in_maps(x, weight)
    res = run_bass_kernel_spmd(nc, in_maps, core_ids=list(range(N_CORES)))
    return gather_output(res.results)
